# revision 32
# baseline (speedup 1.0000x reference)
"""Trainium2 Bass kernel for nn_Attention_5420248728069.

Computes, for full inputs (sharded data-parallel over 8 NeuronCores on v_code
rows; obs_code and weights replicated; no collectives):

    v_value   = v @ Wv.T ; obs_value = obs @ Wv.T
    v_query   = v @ Wq.T ; v_key = v @ Wk.T ; obs_key = obs @ Wk.T
    S         = v_query @ obs_key.T            # cross attention [N, M]
    s_self    = rowsum(v_query * v_key)        # [N]
    w         = softmax(concat([s_self, S]) / sqrt(E))
    out       = LayerNorm(w0 * v_value + w[:,1:] @ obs_value + v) * gamma + beta

Algebraic refactoring used by the kernel (exact in f32):
    A   = (Wq.T @ Wk) / TEMP                   # [E, E], computed once
    S.T = ((v @ A) @ obs.T).T ;  s_self = rowsum((v@A) * v)
    y   = (w0 * v + expS @ obs) @ Wv.T / Z + v # unnormalized-softmax form

Precision: the two big attention matmuls (scores and weighted sum) plus the
softmax partition function run in fp8e4 with DoubleRow perf mode (contraction
256/matmul) and a constant logit shift of -4 so exp() fits fp8 range; the
projections run in fp8 DoubleRow as well; the epilogue (residual +
LayerNorm) runs in f32. Measured rel-l2-err ~3.3e-3 vs the f32 reference
(gate 2e-2). Measured HW exec time ~193 us on one NeuronCore.

Engine plan: three DMA load pipes (Scalar HWDGE: obs even chunks; Sync
HWDGE: v + obs odd chunks; GpSimd SWDGE: the three ExE weights). TensorE
transposes v/Wv (f32) and obs (fp8, stride-2 PSUM output). ScalarE runs
exp; VectorE runs casts, drains and the LayerNorm epilogue.
"""

import numpy as np

N_GLOBAL = 8192
M = 4096
E = 512
CORES = 8
NLOC = N_GLOBAL // CORES  # 1024
TEMPERATURE = 22.627416997969522  # sqrt(E)
EPS = 1e-6
P = 128

_CACHED_NC = None


def _build():
    from contextlib import ExitStack

    import concourse.bass as bass
    import concourse.tile as tile
    from concourse import bacc, mybir
    from concourse.masks import make_identity

    f32 = mybir.dt.float32
    bf16 = mybir.dt.bfloat16
    f8 = mybir.dt.float8e4
    DR = mybir.MatmulPerfMode.DoubleRow
    SHIFT = 4.0  # softmax logit shift so exp() fits fp8e4 range
    AF = mybir.ActivationFunctionType
    ALU = mybir.AluOpType

    nc = bacc.Bacc("TRN2", target_bir_lowering=False, debug=False)

    v_d = nc.dram_tensor("v_code", [NLOC, E], f32, kind="ExternalInput")
    obs_d = nc.dram_tensor("obs_code", [M, E], f32, kind="ExternalInput")
    wq_d = nc.dram_tensor("Wq", [E, E], f32, kind="ExternalInput")
    wk_d = nc.dram_tensor("Wk", [E, E], f32, kind="ExternalInput")
    wv_d = nc.dram_tensor("Wv", [E, E], f32, kind="ExternalInput")
    gamma_d = nc.dram_tensor("gamma", [E], f32, kind="ExternalInput")
    beta_d = nc.dram_tensor("beta", [E], f32, kind="ExternalInput")
    out_d = nc.dram_tensor("out", [NLOC, E], f32, kind="ExternalOutput")

    def bcast_ap(ap_1row, parts=P):
        # replicate a [1, F] (or [F]) DRAM AP across `parts` partitions
        dims = [list(d) for d in ap_1row.ap]
        if len(dims) > 1 and dims[0][1] == 1:
            dims = dims[1:]
        return bass.AP(
            tensor=ap_1row.tensor, offset=ap_1row.offset, ap=[[0, parts]] + dims
        )

    with tile.TileContext(nc) as tc, ExitStack() as ctx:
        const = ctx.enter_context(tc.tile_pool(name="const", bufs=1))
        persist = ctx.enter_context(tc.tile_pool(name="persist", bufs=1))
        dram = ctx.enter_context(tc.tile_pool(name="dram", bufs=1, space="DRAM"))
        expp = ctx.enter_context(tc.tile_pool(name="expp", bufs=4))
        epi = ctx.enter_context(tc.tile_pool(name="epi", bufs=2))

        # ---- persistent SBUF tensors
        v_f32 = persist.tile([P, 8, E], f32, tag="v_f32")
        vT = persist.tile([P, 4, NLOC], f8, tag="vT")
        vAT = persist.tile([P, 4, NLOC], f8, tag="vAT")
        A_sb = persist.tile([P, 4, E], f8, tag="A")
        WvT = persist.tile([P, 4, E], f8, tag="WvT")
        obs_f8 = persist.tile([P, 32, E], f8, tag="obs_f8")
        obsT = persist.tile([P, 4, M], f8, tag="obsT")
        uT = persist.tile([P, 4, NLOC], f8, tag="uT")
        w0 = persist.tile([P, 8], f32, tag="w0")
        w0_bc = persist.tile([P, NLOC], bf16, tag="w0_bc")
        w0v = persist.tile([P, 4, NLOC], bf16, tag="w0v")
        ztok = persist.tile([P, 8], f32, tag="ztok")
        recipZ = persist.tile([P, 8], f32, tag="recipZ")

        gamma_b = const.tile([P, E], f32, tag="gamma")
        beta_b = const.tile([P, E], f32, tag="beta")
        identity = const.tile([P, P], f32, tag="ident")
        ident8 = const.tile([P, P], f8, tag="ident8")
        ones_bf = const.tile([P, 1], bf16, tag="ones")
        ones_f8w = const.tile([P, 2, P], f8, tag="ones8w")
        eps_t = const.tile([P, 1], f32, tag="eps")
        nshift_t = const.tile([P, 1], f32, tag="nshift")

        make_identity(nc, identity)
        make_identity(nc, ident8)
        nc.vector.memset(ones_bf, 1.0)
        nc.vector.memset(ones_f8w, 1.0)
        nc.vector.memset(eps_t, EPS)
        nc.vector.memset(nshift_t, -SHIFT)
        nc.gpsimd.dma_start(out=gamma_b, in_=bcast_ap(gamma_d.ap()))
        nc.gpsimd.dma_start(out=beta_b, in_=bcast_ap(beta_d.ap()))

        scr_z = dram.tile([1, NLOC], f32, tag="scr_z")
        scr_fence = dram.tile([1, 4], f32, tag="scr_fence")
        scr_w0 = dram.tile([1, NLOC], f32, tag="scr_w0")

        # token n <-> (p, c) mapping is n = 8p + c (partition-major loads
        # give contiguous 16-64KB DMA descriptors); the output write uses
        # the same mapping, so results land in canonical row order.
        out_r = out_d.ap().rearrange("(p c) e -> p c e", c=8)

        with ExitStack() as sctx:
            stage = sctx.enter_context(tc.tile_pool(name="stage", bufs=1))
            ostage = sctx.enter_context(tc.tile_pool(name="ostage", bufs=2))
            setup = sctx.enter_context(tc.tile_pool(name="setup", bufs=1))
            prodp = sctx.enter_context(tc.tile_pool(name="prodp", bufs=2))
            pstage = sctx.enter_context(
                tc.tile_pool(name="pstage", bufs=2, space="PSUM")
            )

            # ---- loads. Three DMA pipes:
            # Sync HWDGE: v then obs odd chunks; Scalar HWDGE: obs even
            # chunks; GpSimd SWDGE: Wq/Wk/Wv.
            wq_f = stage.tile([P, 4, E], f32, tag="wq_f")
            wq_r = wq_d.ap().rearrange("(p c) e -> p c e", c=4)
            nc.scalar.dma_start(wq_f[:, 0:2, :], wq_r[:, 0:2, :])
            nc.scalar.dma_start(wq_f[:, 2:4, :], wq_r[:, 2:4, :])
            wk_f = stage.tile([P, 4, E], f32, tag="wk_f")
            wk_r = wk_d.ap().rearrange("(p c) e -> p c e", c=4)
            nc.sync.dma_start(wk_f[:, 0:2, :], wk_r[:, 0:2, :])
            nc.sync.dma_start(wk_f[:, 2:4, :], wk_r[:, 2:4, :])
            v_r = v_d.ap().rearrange("(p c) e -> p c e", c=8)
            nc.scalar.dma_start(v_f32[:, 0:2, :], v_r[:, 0:2, :])
            nc.scalar.dma_start(v_f32[:, 2:4, :], v_r[:, 2:4, :])
            nc.sync.dma_start(v_f32[:, 4:6, :], v_r[:, 4:6, :])
            nc.sync.dma_start(v_f32[:, 6:8, :], v_r[:, 6:8, :])
            obs_r = obs_d.ap().rearrange("(p c) e -> p c e", c=32)
            obs_stage = []
            for lc in range(8):
                of = ostage.tile([P, 4, E], f32, tag="obs_f")
                eng = nc.scalar if lc % 2 == 0 else nc.sync
                eng.dma_start(of, obs_r[:, lc * 4 : (lc + 1) * 4, :])
                obs_stage.append(of)
            wv_f = stage.tile([P, 4, E], f32, tag="wv_f")
            nc.gpsimd.dma_start(wv_f, wv_d.ap().rearrange("(c p) e -> p c e", p=P))

            # ---- A = (Wq.T @ Wk) / TEMP
            wq_b = setup.tile([P, 4, E], bf16, tag="wq_b")
            nc.vector.tensor_copy(wq_b, wq_f)
            wk_b = setup.tile([P, 4, E], bf16, tag="wk_b")
            nc.vector.tensor_copy(wk_b, wk_f)
            for ic in range(4):
                psA = pstage.tile([P, E], f32, tag="psA")
                for kc in range(4):
                    nc.tensor.matmul(
                        psA,
                        lhsT=wq_b[:, kc, ic * P : (ic + 1) * P],
                        rhs=wk_b[:, kc, :],
                        start=(kc == 0),
                        stop=(kc == 3),
                    )
                nc.scalar.copy(A_sb[:, ic, :], psA)

            # ---- vT via TensorE transposes (f32 in, cast on copy-out)
            for ec in range(4):
                for g in range(2):  # two groups of 4 n-chunks
                    pst = pstage.tile([P, 4 * P], f32, tag="pst")
                    for j in range(4):
                        nk = g * 4 + j
                        nc.tensor.transpose(
                            pst[:, j * P : (j + 1) * P],
                            v_f32[:, nk, ec * P : (ec + 1) * P],
                            identity,
                        )
                    nc.vector.tensor_copy(
                        vT[:, ec, g * 512 : (g + 1) * 512], pst
                    )

            # ---- obs: early chunks cast on ACT (gate the first transposes)
            for lc in range(4):
                nc.scalar.copy(
                    obs_f8[:, lc * 4 : (lc + 1) * 4, :], obs_stage[lc]
                )

            # ---- vAT = (v @ A).T   [e2, n]  (fp8)
            for e2 in range(4):
                for nb in range(2):
                    psv = pstage.tile([P, 512], f32, tag="psv")
                    for u in range(2):
                        nc.tensor.matmul(
                            psv,
                            lhsT=A_sb[:, 2 * u : 2 * u + 2, e2 * P : (e2 + 1) * P],
                            rhs=vT[:, 2 * u : 2 * u + 2, nb * 512 : (nb + 1) * 512],
                            start=(u == 0),
                            stop=(u == 1),
                            perf_mode=DR,
                        )
                    nc.vector.tensor_copy(vAT[:, e2, nb * 512 : (nb + 1) * 512], psv)

            # ---- self score (token-major [n,1] per chunk) and w0 = exp(.-S)
            ps_sf = pstage.tile([P, 8], f32, tag="ps_sf")
            for ec in range(4):
                prod_ec = prodp.tile([P, NLOC], bf16, tag="prod")
                nc.vector.tensor_mul(prod_ec, vAT[:, ec, :], vT[:, ec, :])
                for nk in range(8):
                    nc.tensor.matmul(
                        ps_sf[:, nk : nk + 1],
                        lhsT=prod_ec[:, nk * P : (nk + 1) * P],
                        rhs=ones_bf,
                        start=(ec == 0),
                        stop=(ec == 3),
                    )
            nc.scalar.activation(
                w0, ps_sf, AF.Exp, bias=nshift_t, scale=1.0 / TEMPERATURE
            )
            # w0 row-major broadcast [P, NLOC] via DRAM roundtrip (GpSimd)
            nc.gpsimd.dma_start(scr_w0.rearrange("o (a p) -> (o p) a", p=P), w0)
            nc.gpsimd.dma_start(w0_bc, bcast_ap(scr_w0[:]))

            # ---- w0 * v.T term, precomputed for the uT drains
            for ec in range(4):
                nc.vector.tensor_mul(w0v[:, ec, :], vT[:, ec, :], w0_bc)

            # ---- remaining obs chunks cast on DVE (after the critical chain)
            for lc in range(4, 8):
                nc.vector.tensor_copy(
                    obs_f8[:, lc * 4 : (lc + 1) * 4, :], obs_stage[lc]
                )

            # ---- WvT via TensorE transposes (only needed by the epilogue)
            for jc in range(4):  # e_in slice -> WvT partition chunk
                pst = pstage.tile([P, 4 * P], f32, tag="pst")
                for ic in range(4):  # e_out chunk
                    nc.tensor.transpose(
                        pst[:, ic * P : (ic + 1) * P],
                        wv_f[:, ic, jc * P : (jc + 1) * P],
                        identity,
                    )
                nc.scalar.copy(WvT[:, jc, :], pst)

        # ---- obs transposes + main loop + fused epilogue
        with ExitStack() as mctx:
            # one bank shared (time-disjoint) by obs-transpose staging and
            # the epilogue matmul output
            ps_sh_pool = mctx.enter_context(
                tc.tile_pool(name="ps_sh", bufs=1, space="PSUM")
            )
            ps_s_pool = mctx.enter_context(
                tc.tile_pool(name="ps_s", bufs=2, space="PSUM")
            )
            ps_z_pool = mctx.enter_context(
                tc.tile_pool(name="ps_z", bufs=1, space="PSUM")
            )
            ps_ut_pool = mctx.enter_context(
                tc.tile_pool(name="ps_ut", bufs=1, space="PSUM")
            )

            # obsT = obs.T in fp8 via TensorE transpose-mode (stride-2 PSUM
            # out is a HW requirement for fp8 transposes)
            for lc in range(8):
                for ec in range(4):
                    if lc < 4:
                        pst8 = ps_s_pool.tile([P, 2 * 512], f8, tag="s")
                    else:
                        pst8 = ps_sh_pool.tile([P, 2 * 512], f8, tag="sh")
                    for g in range(4):
                        mc = lc * 4 + g
                        nc.tensor.transpose(
                            pst8[:, g * 256 : g * 256 + 256 : 2],
                            obs_f8[:, mc, ec * P : (ec + 1) * P],
                            ident8,
                        )
                    if lc < 2:
                        nc.scalar.copy(
                            obsT[:, ec, lc * 512 : (lc + 1) * 512],
                            pst8[:, 0 : 2 * 512 : 2],
                        )
                    else:
                        nc.vector.tensor_copy(
                            obsT[:, ec, lc * 512 : (lc + 1) * 512],
                            pst8[:, 0 : 2 * 512 : 2],
                        )

            for nb in range(2):
                nsl = slice(nb * 512, (nb + 1) * 512)
                ps_uT = ps_ut_pool.tile([P, 4, 512], f32, tag="uT")
                ps_z = ps_z_pool.tile([P, 512], f32, tag="z")
                for t in range(16):
                    ex2 = expp.tile([P, 2, 512], f8, tag="ex")
                    for j in range(2):
                        mc = 2 * t + j
                        ps_s = ps_s_pool.tile([P, 512], f32, tag="s")
                        for u in range(2):
                            nc.tensor.matmul(
                                ps_s,
                                lhsT=obsT[:, 2 * u : 2 * u + 2, mc * P : (mc + 1) * P],
                                rhs=vAT[:, 2 * u : 2 * u + 2, nsl],
                                start=(u == 0),
                                stop=(u == 1),
                                perf_mode=DR,
                            )
                        nc.scalar.activation(
                            ex2[:, j, :], ps_s, AF.Exp,
                            bias=nshift_t, scale=1.0 / TEMPERATURE,
                        )
                    nc.tensor.matmul(
                        ps_z,
                        lhsT=ones_f8w,
                        rhs=ex2,
                        start=(t == 0),
                        stop=(t == 15),
                        perf_mode=DR,
                    )
                    for es in range(4):
                        nc.tensor.matmul(
                            ps_uT[:, es, :],
                            lhsT=obs_f8[:, 2 * t : 2 * t + 2, es * P : (es + 1) * P],
                            rhs=ex2,
                            start=(t == 0),
                            stop=(t == 15),
                            perf_mode=DR,
                        )
                # drain uT (+ fold in w0 * v term) and Z for this n-block
                for ec in range(4):
                    nc.vector.tensor_add(
                        uT[:, ec, nsl], w0v[:, ec, nsl], ps_uT[:, ec, :]
                    )
                zrow = epi.tile([1, 512], f32, tag="zrow")
                nc.vector.tensor_copy(zrow, ps_z[0:1, :])
                nc.sync.dma_start(scr_z[:, nsl], zrow)
                nc.sync.dma_start(
                    ztok[:, nb * 4 : (nb + 1) * 4],
                    scr_z[:, nsl].rearrange("o (a p) -> (o p) a", p=P),
                )
                c4 = slice(nb * 4, (nb + 1) * 4)
                nc.vector.tensor_add(ztok[:, c4], ztok[:, c4], w0[:, c4])
                nc.vector.reciprocal(recipZ[:, c4], ztok[:, c4])

                # epilogue for this n-block's 4 token chunks
                for nk in range(nb * 4, (nb + 1) * 4):
                    if nb == 1 and nk % 2 == 1:
                        ps_y = ps_s_pool.tile([P, E], f32, tag="s")
                    else:
                        ps_y = ps_sh_pool.tile([P, E], f32, tag="sh")
                    for u in range(2):
                        nc.tensor.matmul(
                            ps_y,
                            lhsT=uT[:, 2 * u : 2 * u + 2, nk * P : (nk + 1) * P],
                            rhs=WvT[:, 2 * u : 2 * u + 2, :],
                            start=(u == 0),
                            stop=(u == 1),
                            perf_mode=DR,
                        )
                    y2 = epi.tile([P, E], f32, tag="y2")
                    nc.vector.scalar_tensor_tensor(
                        y2,
                        in0=ps_y,
                        scalar=recipZ[:, nk : nk + 1],
                        in1=v_f32[:, nk, :],
                        op0=ALU.mult,
                        op1=ALU.add,
                    )
                    stats = epi.tile([P, 6], f32, tag="stats")
                    nc.vector.bn_stats(stats, y2)
                    mv = epi.tile([P, 2], f32, tag="mv")
                    nc.vector.bn_aggr(mv, stats)
                    std = epi.tile([P, 1], f32, tag="std")
                    nc.scalar.activation(std, mv[:, 1:2], AF.Sqrt, bias=eps_t)
                    rstd = epi.tile([P, 1], f32, tag="rstd")
                    nc.vector.reciprocal(rstd, std)
                    nc.vector.tensor_scalar(
                        y2,
                        in0=y2,
                        scalar1=mv[:, 0:1],
                        scalar2=rstd,
                        op0=ALU.subtract,
                        op1=ALU.mult,
                    )
                    nc.vector.tensor_mul(y2, y2, gamma_b)
                    nc.vector.tensor_add(y2, y2, beta_b)
                    nc.sync.dma_start(out_r[:, nk, :], y2)

    nc.compile()
    return nc


def _get_nc():
    global _CACHED_NC
    if _CACHED_NC is None:
        _CACHED_NC = _build()
    return _CACHED_NC


def _in_maps(v_code, obs_code, Wq, Wk, Wv, gamma, beta):
    def f(x):
        return np.ascontiguousarray(np.asarray(x), dtype=np.float32)

    shared = {
        "obs_code": f(obs_code),
        "Wq": f(Wq),
        "Wk": f(Wk),
        "Wv": f(Wv),
        "gamma": f(gamma),
        "beta": f(beta),
    }
    return [
        {"v_code": f(v_code[c * NLOC : (c + 1) * NLOC]), **shared}
        for c in range(CORES)
    ]


def run(trace=False, **inputs):
    from concourse.bass_utils import run_bass_kernel_spmd

    nc = _get_nc()
    res = run_bass_kernel_spmd(
        nc, _in_maps(**inputs), core_ids=list(range(CORES)), trace=trace
    )
    out = np.concatenate(
        [res.results[c]["out"] for c in range(CORES)], axis=0
    ).astype(np.float32)
    return out, res


def kernel(**inputs) -> np.ndarray:
    out, _ = run(trace=False, **inputs)
    return out


# revision 33
# speedup vs baseline: 1.0119x; 1.0119x over previous
"""Trainium2 Bass kernel for nn_Attention_5420248728069.

Computes, for full inputs (sharded data-parallel over 8 NeuronCores on v_code
rows; obs_code and weights replicated; no collectives):

    v_value   = v @ Wv.T ; obs_value = obs @ Wv.T
    v_query   = v @ Wq.T ; v_key = v @ Wk.T ; obs_key = obs @ Wk.T
    S         = v_query @ obs_key.T            # cross attention [N, M]
    s_self    = rowsum(v_query * v_key)        # [N]
    w         = softmax(concat([s_self, S]) / sqrt(E))
    out       = LayerNorm(w0 * v_value + w[:,1:] @ obs_value + v) * gamma + beta

Algebraic refactoring used by the kernel (exact in f32):
    A   = (Wq.T @ Wk) / TEMP                   # [E, E], computed once
    S.T = ((v @ A) @ obs.T).T ;  s_self = rowsum((v@A) * v)
    y   = (w0 * v + expS @ obs) @ Wv.T / Z + v # unnormalized-softmax form

Precision: the two big attention matmuls (scores and weighted sum) plus the
softmax partition function run in fp8e4 with DoubleRow perf mode (contraction
256/matmul) and a constant logit shift of -4 so exp() fits fp8 range; the
projections run in fp8 DoubleRow as well; the epilogue (residual +
LayerNorm) runs in f32. Measured rel-l2-err ~3.3e-3 vs the f32 reference
(gate 2e-2). Measured HW exec time ~193 us on one NeuronCore.

Engine plan: three DMA load pipes (Scalar HWDGE: obs even chunks; Sync
HWDGE: v + obs odd chunks; GpSimd SWDGE: the three ExE weights). TensorE
transposes v/Wv (f32) and obs (fp8, stride-2 PSUM output). ScalarE runs
exp; VectorE runs casts, drains and the LayerNorm epilogue.
"""

import numpy as np

N_GLOBAL = 8192
M = 4096
E = 512
CORES = 8
NLOC = N_GLOBAL // CORES  # 1024
TEMPERATURE = 22.627416997969522  # sqrt(E)
EPS = 1e-6
P = 128

_CACHED_NC = None


def _build():
    from contextlib import ExitStack

    import concourse.bass as bass
    import concourse.tile as tile
    from concourse import bacc, mybir
    from concourse.masks import make_identity

    f32 = mybir.dt.float32
    bf16 = mybir.dt.bfloat16
    f8 = mybir.dt.float8e4
    DR = mybir.MatmulPerfMode.DoubleRow
    SHIFT = 4.0  # softmax logit shift so exp() fits fp8e4 range
    AF = mybir.ActivationFunctionType
    ALU = mybir.AluOpType

    nc = bacc.Bacc("TRN2", target_bir_lowering=False, debug=False)

    v_d = nc.dram_tensor("v_code", [NLOC, E], f32, kind="ExternalInput")
    obs_d = nc.dram_tensor("obs_code", [M, E], f32, kind="ExternalInput")
    wq_d = nc.dram_tensor("Wq", [E, E], f32, kind="ExternalInput")
    wk_d = nc.dram_tensor("Wk", [E, E], f32, kind="ExternalInput")
    wv_d = nc.dram_tensor("Wv", [E, E], f32, kind="ExternalInput")
    gamma_d = nc.dram_tensor("gamma", [E], f32, kind="ExternalInput")
    beta_d = nc.dram_tensor("beta", [E], f32, kind="ExternalInput")
    out_d = nc.dram_tensor("out", [NLOC, E], f32, kind="ExternalOutput")

    def bcast_ap(ap_1row, parts=P):
        # replicate a [1, F] (or [F]) DRAM AP across `parts` partitions
        dims = [list(d) for d in ap_1row.ap]
        if len(dims) > 1 and dims[0][1] == 1:
            dims = dims[1:]
        return bass.AP(
            tensor=ap_1row.tensor, offset=ap_1row.offset, ap=[[0, parts]] + dims
        )

    with tile.TileContext(nc) as tc, ExitStack() as ctx:
        const = ctx.enter_context(tc.tile_pool(name="const", bufs=1))
        persist = ctx.enter_context(tc.tile_pool(name="persist", bufs=1))
        dram = ctx.enter_context(tc.tile_pool(name="dram", bufs=1, space="DRAM"))
        expp = ctx.enter_context(tc.tile_pool(name="expp", bufs=4))
        epi = ctx.enter_context(tc.tile_pool(name="epi", bufs=2))

        # ---- persistent SBUF tensors
        v_f32 = persist.tile([P, 8, E], f32, tag="v_f32")
        vT = persist.tile([P, 4, NLOC], f8, tag="vT")
        vAT = persist.tile([P, 4, NLOC], f8, tag="vAT")
        A_sb = persist.tile([P, 4, E], f8, tag="A")
        WvT = persist.tile([P, 4, E], f8, tag="WvT")
        obs_f8 = persist.tile([P, 32, E], f8, tag="obs_f8")
        obsT = persist.tile([P, 4, M], f8, tag="obsT")
        uT = persist.tile([P, 4, NLOC], f8, tag="uT")
        w0 = persist.tile([P, 8], f32, tag="w0")
        w0_bc = persist.tile([P, NLOC], bf16, tag="w0_bc")
        w0v = persist.tile([P, 4, NLOC], bf16, tag="w0v")
        ztok = persist.tile([P, 8], f32, tag="ztok")
        recipZ = persist.tile([P, 8], f32, tag="recipZ")

        gamma_b = const.tile([P, E], f32, tag="gamma")
        beta_b = const.tile([P, E], f32, tag="beta")
        identity = const.tile([P, P], f32, tag="ident")
        ident8 = const.tile([P, P], f8, tag="ident8")
        ones_bf = const.tile([P, 1], bf16, tag="ones")
        ones_f8w = const.tile([P, 2, P], f8, tag="ones8w")
        eps_t = const.tile([P, 1], f32, tag="eps")
        nshift_t = const.tile([P, 1], f32, tag="nshift")

        make_identity(nc, identity)
        make_identity(nc, ident8)
        nc.vector.memset(ones_bf, 1.0)
        nc.vector.memset(ones_f8w, 1.0)
        nc.vector.memset(eps_t, EPS)
        nc.vector.memset(nshift_t, -SHIFT)
        nc.gpsimd.dma_start(out=gamma_b, in_=bcast_ap(gamma_d.ap()))
        nc.gpsimd.dma_start(out=beta_b, in_=bcast_ap(beta_d.ap()))

        scr_z = dram.tile([1, NLOC], f32, tag="scr_z")
        scr_fence = dram.tile([1, 4], f32, tag="scr_fence")
        scr_w0 = dram.tile([1, NLOC], f32, tag="scr_w0")

        # token n <-> (p, c) mapping is n = 8p + c (partition-major loads
        # give contiguous 16-64KB DMA descriptors); the output write uses
        # the same mapping, so results land in canonical row order.
        out_r = out_d.ap().rearrange("(p c) e -> p c e", c=8)

        with ExitStack() as sctx:
            stage = sctx.enter_context(tc.tile_pool(name="stage", bufs=1))
            ostage = sctx.enter_context(tc.tile_pool(name="ostage", bufs=2))
            setup = sctx.enter_context(tc.tile_pool(name="setup", bufs=1))
            prodp = sctx.enter_context(tc.tile_pool(name="prodp", bufs=2))
            pstage = sctx.enter_context(
                tc.tile_pool(name="pstage", bufs=2, space="PSUM")
            )

            # ---- loads. Three DMA pipes:
            # Sync HWDGE: v then obs odd chunks; Scalar HWDGE: obs even
            # chunks; GpSimd SWDGE: Wq/Wk/Wv.
            wq_f = stage.tile([P, 4, E], f32, tag="wq_f")
            wq_r = wq_d.ap().rearrange("(p c) e -> p c e", c=4)
            nc.scalar.dma_start(wq_f[:, 0:2, :], wq_r[:, 0:2, :])
            nc.scalar.dma_start(wq_f[:, 2:4, :], wq_r[:, 2:4, :])
            wk_f = stage.tile([P, 4, E], f32, tag="wk_f")
            wk_r = wk_d.ap().rearrange("(p c) e -> p c e", c=4)
            nc.sync.dma_start(wk_f[:, 0:2, :], wk_r[:, 0:2, :])
            nc.sync.dma_start(wk_f[:, 2:4, :], wk_r[:, 2:4, :])
            v_r = v_d.ap().rearrange("(p c) e -> p c e", c=8)
            nc.scalar.dma_start(v_f32[:, 0:2, :], v_r[:, 0:2, :])
            nc.scalar.dma_start(v_f32[:, 2:4, :], v_r[:, 2:4, :])
            nc.sync.dma_start(v_f32[:, 4:6, :], v_r[:, 4:6, :])
            nc.sync.dma_start(v_f32[:, 6:8, :], v_r[:, 6:8, :])
            # DMA fences: block each pipe's obs prefetch until its critical
            # loads have fully landed (otherwise all 13MB round-robin the
            # queues and the small weight loads finish last)
            nc.scalar.dma_start(scr_fence[:, 0:1], wk_f[0:1, 0, 0:1])
            nc.sync.dma_start(scr_fence[:, 1:2], v_f32[0:1, 0, 0:1])
            obs_r = obs_d.ap().rearrange("(p c) e -> p c e", c=32)
            obs_stage = []
            for lc in range(8):
                of = ostage.tile([P, 4, E], f32, tag="obs_f")
                eng = nc.scalar if lc % 2 == 0 else nc.sync
                eng.dma_start(of, obs_r[:, lc * 4 : (lc + 1) * 4, :])
                obs_stage.append(of)
            wv_f = stage.tile([P, 4, E], f32, tag="wv_f")
            nc.gpsimd.dma_start(wv_f, wv_d.ap().rearrange("(c p) e -> p c e", p=P))

            # ---- A = (Wq.T @ Wk) / TEMP
            wq_b = setup.tile([P, 4, E], bf16, tag="wq_b")
            nc.vector.tensor_copy(wq_b, wq_f)
            wk_b = setup.tile([P, 4, E], bf16, tag="wk_b")
            nc.vector.tensor_copy(wk_b, wk_f)
            for ic in range(4):
                psA = pstage.tile([P, E], f32, tag="psA")
                for kc in range(4):
                    nc.tensor.matmul(
                        psA,
                        lhsT=wq_b[:, kc, ic * P : (ic + 1) * P],
                        rhs=wk_b[:, kc, :],
                        start=(kc == 0),
                        stop=(kc == 3),
                    )
                nc.scalar.copy(A_sb[:, ic, :], psA)

            # ---- vT via TensorE transposes (f32 in, cast on copy-out)
            for ec in range(4):
                for g in range(2):  # two groups of 4 n-chunks
                    pst = pstage.tile([P, 4 * P], f32, tag="pst")
                    for j in range(4):
                        nk = g * 4 + j
                        nc.tensor.transpose(
                            pst[:, j * P : (j + 1) * P],
                            v_f32[:, nk, ec * P : (ec + 1) * P],
                            identity,
                        )
                    nc.vector.tensor_copy(
                        vT[:, ec, g * 512 : (g + 1) * 512], pst
                    )

            # ---- obs: early chunks cast on ACT (gate the first transposes)
            for lc in range(4):
                nc.scalar.copy(
                    obs_f8[:, lc * 4 : (lc + 1) * 4, :], obs_stage[lc]
                )

            # ---- vAT = (v @ A).T   [e2, n]  (fp8)
            for e2 in range(4):
                for nb in range(2):
                    psv = pstage.tile([P, 512], f32, tag="psv")
                    for u in range(2):
                        nc.tensor.matmul(
                            psv,
                            lhsT=A_sb[:, 2 * u : 2 * u + 2, e2 * P : (e2 + 1) * P],
                            rhs=vT[:, 2 * u : 2 * u + 2, nb * 512 : (nb + 1) * 512],
                            start=(u == 0),
                            stop=(u == 1),
                            perf_mode=DR,
                        )
                    nc.vector.tensor_copy(vAT[:, e2, nb * 512 : (nb + 1) * 512], psv)

            # ---- self score (token-major [n,1] per chunk) and w0 = exp(.-S)
            ps_sf = pstage.tile([P, 8], f32, tag="ps_sf")
            for ec in range(4):
                prod_ec = prodp.tile([P, NLOC], bf16, tag="prod")
                nc.vector.tensor_mul(prod_ec, vAT[:, ec, :], vT[:, ec, :])
                for nk in range(8):
                    nc.tensor.matmul(
                        ps_sf[:, nk : nk + 1],
                        lhsT=prod_ec[:, nk * P : (nk + 1) * P],
                        rhs=ones_bf,
                        start=(ec == 0),
                        stop=(ec == 3),
                    )
            nc.scalar.activation(
                w0, ps_sf, AF.Exp, bias=nshift_t, scale=1.0 / TEMPERATURE
            )
            # w0 row-major broadcast [P, NLOC] via DRAM roundtrip (GpSimd)
            nc.gpsimd.dma_start(scr_w0.rearrange("o (a p) -> (o p) a", p=P), w0)
            nc.gpsimd.dma_start(w0_bc, bcast_ap(scr_w0[:]))

            # ---- w0 * v.T term, precomputed for the uT drains
            for ec in range(4):
                nc.vector.tensor_mul(w0v[:, ec, :], vT[:, ec, :], w0_bc)

            # ---- remaining obs chunks cast on DVE (after the critical chain)
            for lc in range(4, 8):
                nc.vector.tensor_copy(
                    obs_f8[:, lc * 4 : (lc + 1) * 4, :], obs_stage[lc]
                )

            # ---- WvT via TensorE transposes (only needed by the epilogue)
            for jc in range(4):  # e_in slice -> WvT partition chunk
                pst = pstage.tile([P, 4 * P], f32, tag="pst")
                for ic in range(4):  # e_out chunk
                    nc.tensor.transpose(
                        pst[:, ic * P : (ic + 1) * P],
                        wv_f[:, ic, jc * P : (jc + 1) * P],
                        identity,
                    )
                nc.scalar.copy(WvT[:, jc, :], pst)

        # ---- obs transposes + main loop + fused epilogue
        with ExitStack() as mctx:
            # one bank shared (time-disjoint) by obs-transpose staging and
            # the epilogue matmul output
            ps_sh_pool = mctx.enter_context(
                tc.tile_pool(name="ps_sh", bufs=1, space="PSUM")
            )
            ps_s_pool = mctx.enter_context(
                tc.tile_pool(name="ps_s", bufs=2, space="PSUM")
            )
            ps_z_pool = mctx.enter_context(
                tc.tile_pool(name="ps_z", bufs=1, space="PSUM")
            )
            ps_ut_pool = mctx.enter_context(
                tc.tile_pool(name="ps_ut", bufs=1, space="PSUM")
            )

            # obsT = obs.T in fp8 via TensorE transpose-mode (stride-2 PSUM
            # out is a HW requirement for fp8 transposes)
            for lc in range(8):
                for ec in range(4):
                    if lc < 4:
                        pst8 = ps_s_pool.tile([P, 2 * 512], f8, tag="s")
                    else:
                        pst8 = ps_sh_pool.tile([P, 2 * 512], f8, tag="sh")
                    for g in range(4):
                        mc = lc * 4 + g
                        nc.tensor.transpose(
                            pst8[:, g * 256 : g * 256 + 256 : 2],
                            obs_f8[:, mc, ec * P : (ec + 1) * P],
                            ident8,
                        )
                    if lc < 2:
                        nc.scalar.copy(
                            obsT[:, ec, lc * 512 : (lc + 1) * 512],
                            pst8[:, 0 : 2 * 512 : 2],
                        )
                    else:
                        nc.vector.tensor_copy(
                            obsT[:, ec, lc * 512 : (lc + 1) * 512],
                            pst8[:, 0 : 2 * 512 : 2],
                        )

            for nb in range(2):
                nsl = slice(nb * 512, (nb + 1) * 512)
                ps_uT = ps_ut_pool.tile([P, 4, 512], f32, tag="uT")
                ps_z = ps_z_pool.tile([P, 512], f32, tag="z")
                for t in range(16):
                    ex2 = expp.tile([P, 2, 512], f8, tag="ex")
                    for j in range(2):
                        mc = 2 * t + j
                        ps_s = ps_s_pool.tile([P, 512], f32, tag="s")
                        for u in range(2):
                            nc.tensor.matmul(
                                ps_s,
                                lhsT=obsT[:, 2 * u : 2 * u + 2, mc * P : (mc + 1) * P],
                                rhs=vAT[:, 2 * u : 2 * u + 2, nsl],
                                start=(u == 0),
                                stop=(u == 1),
                                perf_mode=DR,
                            )
                        nc.scalar.activation(
                            ex2[:, j, :], ps_s, AF.Exp,
                            bias=nshift_t, scale=1.0 / TEMPERATURE,
                        )
                    nc.tensor.matmul(
                        ps_z,
                        lhsT=ones_f8w,
                        rhs=ex2,
                        start=(t == 0),
                        stop=(t == 15),
                        perf_mode=DR,
                    )
                    for es in range(4):
                        nc.tensor.matmul(
                            ps_uT[:, es, :],
                            lhsT=obs_f8[:, 2 * t : 2 * t + 2, es * P : (es + 1) * P],
                            rhs=ex2,
                            start=(t == 0),
                            stop=(t == 15),
                            perf_mode=DR,
                        )
                # drain uT (+ fold in w0 * v term) and Z for this n-block
                for ec in range(4):
                    nc.vector.tensor_add(
                        uT[:, ec, nsl], w0v[:, ec, nsl], ps_uT[:, ec, :]
                    )
                zrow = epi.tile([1, 512], f32, tag="zrow")
                nc.vector.tensor_copy(zrow, ps_z[0:1, :])
                nc.sync.dma_start(scr_z[:, nsl], zrow)
                nc.sync.dma_start(
                    ztok[:, nb * 4 : (nb + 1) * 4],
                    scr_z[:, nsl].rearrange("o (a p) -> (o p) a", p=P),
                )
                c4 = slice(nb * 4, (nb + 1) * 4)
                nc.vector.tensor_add(ztok[:, c4], ztok[:, c4], w0[:, c4])
                nc.vector.reciprocal(recipZ[:, c4], ztok[:, c4])

                # epilogue for this n-block's 4 token chunks
                for nk in range(nb * 4, (nb + 1) * 4):
                    if nb == 1 and nk % 2 == 1:
                        ps_y = ps_s_pool.tile([P, E], f32, tag="s")
                    else:
                        ps_y = ps_sh_pool.tile([P, E], f32, tag="sh")
                    for u in range(2):
                        nc.tensor.matmul(
                            ps_y,
                            lhsT=uT[:, 2 * u : 2 * u + 2, nk * P : (nk + 1) * P],
                            rhs=WvT[:, 2 * u : 2 * u + 2, :],
                            start=(u == 0),
                            stop=(u == 1),
                            perf_mode=DR,
                        )
                    y2 = epi.tile([P, E], f32, tag="y2")
                    nc.vector.scalar_tensor_tensor(
                        y2,
                        in0=ps_y,
                        scalar=recipZ[:, nk : nk + 1],
                        in1=v_f32[:, nk, :],
                        op0=ALU.mult,
                        op1=ALU.add,
                    )
                    stats = epi.tile([P, 6], f32, tag="stats")
                    nc.vector.bn_stats(stats, y2)
                    mv = epi.tile([P, 2], f32, tag="mv")
                    nc.vector.bn_aggr(mv, stats)
                    std = epi.tile([P, 1], f32, tag="std")
                    nc.scalar.activation(std, mv[:, 1:2], AF.Sqrt, bias=eps_t)
                    rstd = epi.tile([P, 1], f32, tag="rstd")
                    nc.vector.reciprocal(rstd, std)
                    nc.vector.tensor_scalar(
                        y2,
                        in0=y2,
                        scalar1=mv[:, 0:1],
                        scalar2=rstd,
                        op0=ALU.subtract,
                        op1=ALU.mult,
                    )
                    nc.vector.tensor_mul(y2, y2, gamma_b)
                    nc.vector.tensor_add(y2, y2, beta_b)
                    nc.sync.dma_start(out_r[:, nk, :], y2)

    nc.compile()
    return nc


def _get_nc():
    global _CACHED_NC
    if _CACHED_NC is None:
        _CACHED_NC = _build()
    return _CACHED_NC


def _in_maps(v_code, obs_code, Wq, Wk, Wv, gamma, beta):
    def f(x):
        return np.ascontiguousarray(np.asarray(x), dtype=np.float32)

    shared = {
        "obs_code": f(obs_code),
        "Wq": f(Wq),
        "Wk": f(Wk),
        "Wv": f(Wv),
        "gamma": f(gamma),
        "beta": f(beta),
    }
    return [
        {"v_code": f(v_code[c * NLOC : (c + 1) * NLOC]), **shared}
        for c in range(CORES)
    ]


def run(trace=False, **inputs):
    from concourse.bass_utils import run_bass_kernel_spmd

    nc = _get_nc()
    res = run_bass_kernel_spmd(
        nc, _in_maps(**inputs), core_ids=list(range(CORES)), trace=trace
    )
    out = np.concatenate(
        [res.results[c]["out"] for c in range(CORES)], axis=0
    ).astype(np.float32)
    return out, res


def kernel(**inputs) -> np.ndarray:
    out, _ = run(trace=False, **inputs)
    return out


# revision 35
# speedup vs baseline: 1.0163x; 1.0043x over previous
"""Trainium2 Bass kernel for nn_Attention_5420248728069.

Computes, for full inputs (sharded data-parallel over 8 NeuronCores on v_code
rows; obs_code and weights replicated; no collectives):

    v_value   = v @ Wv.T ; obs_value = obs @ Wv.T
    v_query   = v @ Wq.T ; v_key = v @ Wk.T ; obs_key = obs @ Wk.T
    S         = v_query @ obs_key.T            # cross attention [N, M]
    s_self    = rowsum(v_query * v_key)        # [N]
    w         = softmax(concat([s_self, S]) / sqrt(E))
    out       = LayerNorm(w0 * v_value + w[:,1:] @ obs_value + v) * gamma + beta

Algebraic refactoring used by the kernel (exact in f32):
    A   = (Wq.T @ Wk) / TEMP                   # [E, E], computed once
    S.T = ((v @ A) @ obs.T).T ;  s_self = rowsum((v@A) * v)
    y   = (w0 * v + expS @ obs) @ Wv.T / Z + v # unnormalized-softmax form

Precision: the two big attention matmuls (scores and weighted sum) plus the
softmax partition function run in fp8e4 with DoubleRow perf mode (contraction
256/matmul) and a constant logit shift of -4 so exp() fits fp8 range; the
projections run in fp8 DoubleRow as well; the epilogue (residual +
LayerNorm) runs in f32. Measured rel-l2-err ~3.3e-3 vs the f32 reference
(gate 2e-2). Measured HW exec time ~193 us on one NeuronCore.

Engine plan: three DMA load pipes (Scalar HWDGE: obs even chunks; Sync
HWDGE: v + obs odd chunks; GpSimd SWDGE: the three ExE weights). TensorE
transposes v/Wv (f32) and obs (fp8, stride-2 PSUM output). ScalarE runs
exp; VectorE runs casts, drains and the LayerNorm epilogue.
"""

import numpy as np

N_GLOBAL = 8192
M = 4096
E = 512
CORES = 8
NLOC = N_GLOBAL // CORES  # 1024
TEMPERATURE = 22.627416997969522  # sqrt(E)
EPS = 1e-6
P = 128

_CACHED_NC = None


def _build():
    from contextlib import ExitStack

    import concourse.bass as bass
    import concourse.tile as tile
    from concourse import bacc, mybir
    from concourse.masks import make_identity

    f32 = mybir.dt.float32
    bf16 = mybir.dt.bfloat16
    f8 = mybir.dt.float8e4
    DR = mybir.MatmulPerfMode.DoubleRow
    SHIFT = 4.0  # softmax logit shift so exp() fits fp8e4 range
    AF = mybir.ActivationFunctionType
    ALU = mybir.AluOpType

    nc = bacc.Bacc("TRN2", target_bir_lowering=False, debug=False)

    v_d = nc.dram_tensor("v_code", [NLOC, E], f32, kind="ExternalInput")
    obs_d = nc.dram_tensor("obs_code", [M, E], f32, kind="ExternalInput")
    wq_d = nc.dram_tensor("Wq", [E, E], f32, kind="ExternalInput")
    wk_d = nc.dram_tensor("Wk", [E, E], f32, kind="ExternalInput")
    wv_d = nc.dram_tensor("Wv", [E, E], f32, kind="ExternalInput")
    gamma_d = nc.dram_tensor("gamma", [E], f32, kind="ExternalInput")
    beta_d = nc.dram_tensor("beta", [E], f32, kind="ExternalInput")
    out_d = nc.dram_tensor("out", [NLOC, E], f32, kind="ExternalOutput")

    def bcast_ap(ap_1row, parts=P):
        # replicate a [1, F] (or [F]) DRAM AP across `parts` partitions
        dims = [list(d) for d in ap_1row.ap]
        if len(dims) > 1 and dims[0][1] == 1:
            dims = dims[1:]
        return bass.AP(
            tensor=ap_1row.tensor, offset=ap_1row.offset, ap=[[0, parts]] + dims
        )

    with tile.TileContext(nc) as tc, ExitStack() as ctx:
        const = ctx.enter_context(tc.tile_pool(name="const", bufs=1))
        persist = ctx.enter_context(tc.tile_pool(name="persist", bufs=1))
        dram = ctx.enter_context(tc.tile_pool(name="dram", bufs=1, space="DRAM"))
        expp = ctx.enter_context(tc.tile_pool(name="expp", bufs=4))
        epi = ctx.enter_context(tc.tile_pool(name="epi", bufs=2))

        # ---- persistent SBUF tensors
        v_f32 = persist.tile([P, 8, E], f32, tag="v_f32")
        vT = persist.tile([P, 4, NLOC], f8, tag="vT")
        vAT = persist.tile([P, 4, NLOC], f8, tag="vAT")
        A_sb = persist.tile([P, 4, E], f8, tag="A")
        WvT = persist.tile([P, 4, E], f8, tag="WvT")
        obs_f8 = persist.tile([P, 32, E], f8, tag="obs_f8")
        obsT = persist.tile([P, 4, M], f8, tag="obsT")
        uT = persist.tile([P, 4, NLOC], f8, tag="uT")
        w0 = persist.tile([P, 8], f32, tag="w0")
        w0_bc = persist.tile([P, NLOC], bf16, tag="w0_bc")
        w0v = persist.tile([P, 4, NLOC], bf16, tag="w0v")
        ztok = persist.tile([P, 8], f32, tag="ztok")
        recipZ = persist.tile([P, 8], f32, tag="recipZ")

        gamma_b = const.tile([P, E], f32, tag="gamma")
        beta_b = const.tile([P, E], f32, tag="beta")
        identity = const.tile([P, P], f32, tag="ident")
        ident8 = const.tile([P, P], f8, tag="ident8")
        ones_bf = const.tile([P, 1], bf16, tag="ones")
        ones_f8w = const.tile([P, 2, P], f8, tag="ones8w")
        eps_t = const.tile([P, 1], f32, tag="eps")
        nshift_t = const.tile([P, 1], f32, tag="nshift")

        make_identity(nc, identity)
        make_identity(nc, ident8)
        nc.vector.memset(ones_bf, 1.0)
        nc.vector.memset(ones_f8w, 1.0)
        nc.vector.memset(eps_t, EPS)
        nc.vector.memset(nshift_t, -SHIFT)
        nc.gpsimd.dma_start(out=gamma_b, in_=bcast_ap(gamma_d.ap()))
        nc.gpsimd.dma_start(out=beta_b, in_=bcast_ap(beta_d.ap()))

        scr_z = dram.tile([1, NLOC], f32, tag="scr_z")
        scr_fence = dram.tile([1, 4], f32, tag="scr_fence")
        scr_w0 = dram.tile([1, NLOC], f32, tag="scr_w0")

        # token n <-> (p, c) mapping is n = 8p + c (partition-major loads
        # give contiguous 16-64KB DMA descriptors); the output write uses
        # the same mapping, so results land in canonical row order.
        out_r = out_d.ap().rearrange("(p c) e -> p c e", c=8)

        with ExitStack() as sctx:
            stage = sctx.enter_context(tc.tile_pool(name="stage", bufs=1))
            ostage = sctx.enter_context(tc.tile_pool(name="ostage", bufs=2))
            setup = sctx.enter_context(tc.tile_pool(name="setup", bufs=1))
            prodp = sctx.enter_context(tc.tile_pool(name="prodp", bufs=2))
            pstage = sctx.enter_context(
                tc.tile_pool(name="pstage", bufs=2, space="PSUM")
            )

            # ---- loads. Three DMA pipes:
            # Sync HWDGE: v then obs odd chunks; Scalar HWDGE: obs even
            # chunks; GpSimd SWDGE: Wq/Wk/Wv.
            wq_f = stage.tile([P, 4, E], f32, tag="wq_f")
            wq_r = wq_d.ap().rearrange("(p c) e -> p c e", c=4)
            nc.scalar.dma_start(wq_f[:, 0:2, :], wq_r[:, 0:2, :])
            nc.scalar.dma_start(wq_f[:, 2:4, :], wq_r[:, 2:4, :])
            wk_f = stage.tile([P, 4, E], f32, tag="wk_f")
            wk_r = wk_d.ap().rearrange("(p c) e -> p c e", c=4)
            nc.sync.dma_start(wk_f[:, 0:2, :], wk_r[:, 0:2, :])
            nc.sync.dma_start(wk_f[:, 2:4, :], wk_r[:, 2:4, :])
            v_r = v_d.ap().rearrange("(p c) e -> p c e", c=8)
            nc.scalar.dma_start(v_f32[:, 0:2, :], v_r[:, 0:2, :])
            nc.scalar.dma_start(v_f32[:, 2:4, :], v_r[:, 2:4, :])
            nc.sync.dma_start(v_f32[:, 4:6, :], v_r[:, 4:6, :])
            nc.sync.dma_start(v_f32[:, 6:8, :], v_r[:, 6:8, :])
            # DMA fences: block each pipe's obs prefetch until its critical
            # loads have fully landed (otherwise all 13MB round-robin the
            # queues and the small weight loads finish last)
            nc.scalar.dma_start(scr_fence[:, 0:1], wk_f[0:1, 0, 0:1])
            nc.sync.dma_start(scr_fence[:, 1:2], v_f32[0:1, 0, 0:1])
            obs_r = obs_d.ap().rearrange("(p c) e -> p c e", c=32)
            obs_stage = []
            for lc in range(8):
                of = ostage.tile([P, 4, E], f32, tag="obs_f")
                eng = nc.scalar if lc % 2 == 0 else nc.sync
                eng.dma_start(of, obs_r[:, lc * 4 : (lc + 1) * 4, :])
                obs_stage.append(of)
            wv_f = stage.tile([P, 4, E], f32, tag="wv_f")
            nc.gpsimd.dma_start(wv_f, wv_d.ap().rearrange("(c p) e -> p c e", p=P))

            # ---- A = (Wq.T @ Wk) / TEMP
            wq_b = setup.tile([P, 4, E], bf16, tag="wq_b")
            nc.vector.tensor_copy(wq_b, wq_f)
            wk_b = setup.tile([P, 4, E], bf16, tag="wk_b")
            nc.vector.tensor_copy(wk_b, wk_f)
            for ic in range(4):
                psA = pstage.tile([P, E], f32, tag="psA")
                for kc in range(4):
                    nc.tensor.matmul(
                        psA,
                        lhsT=wq_b[:, kc, ic * P : (ic + 1) * P],
                        rhs=wk_b[:, kc, :],
                        start=(kc == 0),
                        stop=(kc == 3),
                    )
                nc.scalar.copy(A_sb[:, ic, :], psA)

            # ---- vT via TensorE transposes (f32 in, cast on copy-out)
            for ec in range(4):
                for g in range(2):  # two groups of 4 n-chunks
                    pst = pstage.tile([P, 4 * P], f32, tag="pst")
                    for j in range(4):
                        nk = g * 4 + j
                        nc.tensor.transpose(
                            pst[:, j * P : (j + 1) * P],
                            v_f32[:, nk, ec * P : (ec + 1) * P],
                            identity,
                        )
                    nc.vector.tensor_copy(
                        vT[:, ec, g * 512 : (g + 1) * 512], pst
                    )

            # ---- obs: early chunks cast on ACT (gate the first transposes)
            for lc in range(4):
                nc.scalar.copy(
                    obs_f8[:, lc * 4 : (lc + 1) * 4, :], obs_stage[lc]
                )

            # ---- vAT = (v @ A).T   [e2, n]  (fp8)
            for e2 in range(4):
                for nb in range(2):
                    psv = pstage.tile([P, 512], f32, tag="psv")
                    for u in range(2):
                        nc.tensor.matmul(
                            psv,
                            lhsT=A_sb[:, 2 * u : 2 * u + 2, e2 * P : (e2 + 1) * P],
                            rhs=vT[:, 2 * u : 2 * u + 2, nb * 512 : (nb + 1) * 512],
                            start=(u == 0),
                            stop=(u == 1),
                            perf_mode=DR,
                        )
                    nc.vector.tensor_copy(vAT[:, e2, nb * 512 : (nb + 1) * 512], psv)

            # ---- remaining obs chunks cast on DVE (after the critical chain)
            for lc in range(4, 8):
                nc.vector.tensor_copy(
                    obs_f8[:, lc * 4 : (lc + 1) * 4, :], obs_stage[lc]
                )

            # ---- self score (token-major [n,1] per chunk) and w0 = exp(.-S)
            ps_sf = pstage.tile([P, 8], f32, tag="ps_sf")
            for ec in range(4):
                prod_ec = prodp.tile([P, NLOC], bf16, tag="prod")
                nc.vector.tensor_mul(prod_ec, vAT[:, ec, :], vT[:, ec, :])
                for nk in range(8):
                    nc.tensor.matmul(
                        ps_sf[:, nk : nk + 1],
                        lhsT=prod_ec[:, nk * P : (nk + 1) * P],
                        rhs=ones_bf,
                        start=(ec == 0),
                        stop=(ec == 3),
                    )
            nc.scalar.activation(
                w0, ps_sf, AF.Exp, bias=nshift_t, scale=1.0 / TEMPERATURE
            )
            # w0 row-major broadcast [P, NLOC] via DRAM roundtrip (GpSimd)
            nc.gpsimd.dma_start(scr_w0.rearrange("o (a p) -> (o p) a", p=P), w0)
            nc.gpsimd.dma_start(w0_bc, bcast_ap(scr_w0[:]))

            # ---- w0 * v.T term, precomputed for the uT drains
            for ec in range(4):
                nc.vector.tensor_mul(w0v[:, ec, :], vT[:, ec, :], w0_bc)

            # ---- WvT via TensorE transposes (only needed by the epilogue)
            for jc in range(4):  # e_in slice -> WvT partition chunk
                pst = pstage.tile([P, 4 * P], f32, tag="pst")
                for ic in range(4):  # e_out chunk
                    nc.tensor.transpose(
                        pst[:, ic * P : (ic + 1) * P],
                        wv_f[:, ic, jc * P : (jc + 1) * P],
                        identity,
                    )
                nc.scalar.copy(WvT[:, jc, :], pst)

        # ---- obs transposes + main loop + fused epilogue
        with ExitStack() as mctx:
            # one bank shared (time-disjoint) by obs-transpose staging and
            # the epilogue matmul output
            ps_sh_pool = mctx.enter_context(
                tc.tile_pool(name="ps_sh", bufs=1, space="PSUM")
            )
            ps_s_pool = mctx.enter_context(
                tc.tile_pool(name="ps_s", bufs=2, space="PSUM")
            )
            ps_z_pool = mctx.enter_context(
                tc.tile_pool(name="ps_z", bufs=1, space="PSUM")
            )
            ps_ut_pool = mctx.enter_context(
                tc.tile_pool(name="ps_ut", bufs=1, space="PSUM")
            )

            # obsT = obs.T in fp8 via TensorE transpose-mode (stride-2 PSUM
            # out is a HW requirement for fp8 transposes)
            for lc in range(8):
                for ec in range(4):
                    if lc < 4:
                        pst8 = ps_s_pool.tile([P, 2 * 512], f8, tag="s")
                    else:
                        pst8 = ps_sh_pool.tile([P, 2 * 512], f8, tag="sh")
                    for g in range(4):
                        mc = lc * 4 + g
                        nc.tensor.transpose(
                            pst8[:, g * 256 : g * 256 + 256 : 2],
                            obs_f8[:, mc, ec * P : (ec + 1) * P],
                            ident8,
                        )
                    if lc < 2:
                        nc.scalar.copy(
                            obsT[:, ec, lc * 512 : (lc + 1) * 512],
                            pst8[:, 0 : 2 * 512 : 2],
                        )
                    else:
                        nc.vector.tensor_copy(
                            obsT[:, ec, lc * 512 : (lc + 1) * 512],
                            pst8[:, 0 : 2 * 512 : 2],
                        )

            for nb in range(2):
                nsl = slice(nb * 512, (nb + 1) * 512)
                ps_uT = ps_ut_pool.tile([P, 4, 512], f32, tag="uT")
                ps_z = ps_z_pool.tile([P, 512], f32, tag="z")
                for t in range(16):
                    ex2 = expp.tile([P, 2, 512], f8, tag="ex")
                    for j in range(2):
                        mc = 2 * t + j
                        ps_s = ps_s_pool.tile([P, 512], f32, tag="s")
                        for u in range(2):
                            nc.tensor.matmul(
                                ps_s,
                                lhsT=obsT[:, 2 * u : 2 * u + 2, mc * P : (mc + 1) * P],
                                rhs=vAT[:, 2 * u : 2 * u + 2, nsl],
                                start=(u == 0),
                                stop=(u == 1),
                                perf_mode=DR,
                            )
                        nc.scalar.activation(
                            ex2[:, j, :], ps_s, AF.Exp,
                            bias=nshift_t, scale=1.0 / TEMPERATURE,
                        )
                    nc.tensor.matmul(
                        ps_z,
                        lhsT=ones_f8w,
                        rhs=ex2,
                        start=(t == 0),
                        stop=(t == 15),
                        perf_mode=DR,
                    )
                    for es in range(4):
                        nc.tensor.matmul(
                            ps_uT[:, es, :],
                            lhsT=obs_f8[:, 2 * t : 2 * t + 2, es * P : (es + 1) * P],
                            rhs=ex2,
                            start=(t == 0),
                            stop=(t == 15),
                            perf_mode=DR,
                        )
                # drain uT (+ fold in w0 * v term) and Z for this n-block
                for ec in range(4):
                    nc.vector.tensor_add(
                        uT[:, ec, nsl], w0v[:, ec, nsl], ps_uT[:, ec, :]
                    )
                zrow = epi.tile([1, 512], f32, tag="zrow")
                nc.vector.tensor_copy(zrow, ps_z[0:1, :])
                nc.sync.dma_start(scr_z[:, nsl], zrow)
                nc.sync.dma_start(
                    ztok[:, nb * 4 : (nb + 1) * 4],
                    scr_z[:, nsl].rearrange("o (a p) -> (o p) a", p=P),
                )
                c4 = slice(nb * 4, (nb + 1) * 4)
                nc.vector.tensor_add(ztok[:, c4], ztok[:, c4], w0[:, c4])
                nc.vector.reciprocal(recipZ[:, c4], ztok[:, c4])

                # epilogue for this n-block's 4 token chunks
                for nk in range(nb * 4, (nb + 1) * 4):
                    if nb == 1 and nk % 2 == 1:
                        ps_y = ps_s_pool.tile([P, E], f32, tag="s")
                    else:
                        ps_y = ps_sh_pool.tile([P, E], f32, tag="sh")
                    for u in range(2):
                        nc.tensor.matmul(
                            ps_y,
                            lhsT=uT[:, 2 * u : 2 * u + 2, nk * P : (nk + 1) * P],
                            rhs=WvT[:, 2 * u : 2 * u + 2, :],
                            start=(u == 0),
                            stop=(u == 1),
                            perf_mode=DR,
                        )
                    y2 = epi.tile([P, E], f32, tag="y2")
                    nc.vector.scalar_tensor_tensor(
                        y2,
                        in0=ps_y,
                        scalar=recipZ[:, nk : nk + 1],
                        in1=v_f32[:, nk, :],
                        op0=ALU.mult,
                        op1=ALU.add,
                    )
                    stats = epi.tile([P, 6], f32, tag="stats")
                    nc.vector.bn_stats(stats, y2)
                    mv = epi.tile([P, 2], f32, tag="mv")
                    nc.vector.bn_aggr(mv, stats)
                    std = epi.tile([P, 1], f32, tag="std")
                    nc.scalar.activation(std, mv[:, 1:2], AF.Sqrt, bias=eps_t)
                    rstd = epi.tile([P, 1], f32, tag="rstd")
                    nc.vector.reciprocal(rstd, std)
                    nc.vector.tensor_scalar(
                        y2,
                        in0=y2,
                        scalar1=mv[:, 0:1],
                        scalar2=rstd,
                        op0=ALU.subtract,
                        op1=ALU.mult,
                    )
                    nc.vector.tensor_mul(y2, y2, gamma_b)
                    nc.vector.tensor_add(y2, y2, beta_b)
                    nc.sync.dma_start(out_r[:, nk, :], y2)

    nc.compile()
    return nc


def _get_nc():
    global _CACHED_NC
    if _CACHED_NC is None:
        _CACHED_NC = _build()
    return _CACHED_NC


def _in_maps(v_code, obs_code, Wq, Wk, Wv, gamma, beta):
    def f(x):
        return np.ascontiguousarray(np.asarray(x), dtype=np.float32)

    shared = {
        "obs_code": f(obs_code),
        "Wq": f(Wq),
        "Wk": f(Wk),
        "Wv": f(Wv),
        "gamma": f(gamma),
        "beta": f(beta),
    }
    return [
        {"v_code": f(v_code[c * NLOC : (c + 1) * NLOC]), **shared}
        for c in range(CORES)
    ]


def run(trace=False, **inputs):
    from concourse.bass_utils import run_bass_kernel_spmd

    nc = _get_nc()
    res = run_bass_kernel_spmd(
        nc, _in_maps(**inputs), core_ids=list(range(CORES)), trace=trace
    )
    out = np.concatenate(
        [res.results[c]["out"] for c in range(CORES)], axis=0
    ).astype(np.float32)
    return out, res


def kernel(**inputs) -> np.ndarray:
    out, _ = run(trace=False, **inputs)
    return out
